# revision 12
# baseline (speedup 1.0000x reference)
"""Trainium2 Bass kernel for edge-featured multi-head attention (GNN message passing).

Math (per batch b, query i):
  q = nodes@Wq + bq; k = nodes@Wk (bk cancels in softmax); e_ij = edges@We + be
  sim[h,j] = (q_ih . k_jh + q_ih . e_ijh) * DH^-0.5
  attn = softmax_j(sim);  out = concat_h(sum_j attn (v_j + e_ij)) @ Wo + bo

Reformulation used here (edges are never projected to e):
  q.e_ij  = sum_c edges[i,j,c] * qW[i,h,c],  qW[i,h,:] = sum_d q[i,h,d] We[:,h*32+d]
            (+ q.be term, constant over j -> cancels in softmax)
  sum_j attn e_ij -> aE[i,h,c] = sum_j attn[i,h,j] edges[i,j,c]; its contribution to
  the final output is sum_{h,c} aE[i,h,c] (We@Wo)_h[c,m]  (head-block product,
  precomputed on host).  v path: sum_j attn[i,h,j] vWo_h[j,m] with
  vWo_h = nodes @ (Wkv_v @ Wo)_h.  All constant bias terms fold into one bias_final.

Sharding: i-axis split across 8 cores (64 i per batch per core).  Each core reads
only its 64MB slice of edges (bf16: 2x 32MB in the two layouts it needs).
"""
import numpy as np
import ml_dtypes

B, N, DIM = 2, 512, 256
H, DH = 8, 32
INNER = H * DH
NCORES = 8
NI = N // NCORES           # 64 i-rows per batch per core
ROWS = B * NI              # 128 rows per core
SCALE = DH ** -0.5

bf16 = ml_dtypes.bfloat16

_CACHE = {}


def _build_core():
    """Build the per-core Bass program (SPMD: same program, different data)."""
    import concourse.bass as bass
    import concourse.tile as tile
    from concourse import bacc, mybir

    dt = mybir.dt
    AF = mybir.ActivationFunctionType

    nc = bacc.Bacc("TRN2", target_bir_lowering=False)
    from concourse.bass import _add_dep_helper

    def chain(mms):
        for a, b in zip(mms[1:], mms[:-1]):
            _add_dep_helper(a.ins, b.ins, sync=True, reason="psum chain order")

    # ---- DRAM I/O -------------------------------------------------------
    E_NAT = nc.dram_tensor("e_nat", [ROWS, N, DIM], dt.bfloat16, kind="ExternalInput")
    E_TRP = nc.dram_tensor("e_trp", [ROWS, DIM, N], dt.bfloat16, kind="ExternalInput")
    NODT = nc.dram_tensor("nodt", [DIM, B * N], dt.bfloat16, kind="ExternalInput")
    NODTM = nc.dram_tensor("nodtm", [DIM, ROWS], dt.bfloat16, kind="ExternalInput")
    WQ = nc.dram_tensor("wq", [DIM, INNER], dt.bfloat16, kind="ExternalInput")
    WK = nc.dram_tensor("wk", [DIM, INNER], dt.bfloat16, kind="ExternalInput")
    WETD = nc.dram_tensor("wetd", [DH, H * DIM], dt.bfloat16, kind="ExternalInput")
    WKVWO = nc.dram_tensor("wkvwo", [DIM, H * DIM], dt.bfloat16, kind="ExternalInput")
    WEWO = nc.dram_tensor("wewo", [DIM, H * DIM], dt.bfloat16, kind="ExternalInput")
    BQ = nc.dram_tensor("bq", [INNER, 1], dt.float32, kind="ExternalInput")
    BIASF = nc.dram_tensor("biasf", [1, DIM], dt.bfloat16, kind="ExternalInput")
    IDN = nc.dram_tensor("idn", [128, 128], dt.bfloat16, kind="ExternalInput")
    ONESC = nc.dram_tensor("onesc", [128, 1], dt.bfloat16, kind="ExternalInput")
    ONES1F = nc.dram_tensor("ones1f", [1, 128], dt.float32, kind="ExternalInput")
    ONES64 = nc.dram_tensor("ones64", [1, NI], dt.bfloat16, kind="ExternalInput")
    OUT = nc.dram_tensor("out", [ROWS, DIM], dt.float32, kind="ExternalOutput")

    NG = 8          # i-groups per batch
    GI = NI // NG   # 8 i per group
    NT = N // 128   # 4 j-tiles of 128

    with tile.TileContext(nc) as tc:
        with (
            tc.tile_pool(name="wp", bufs=1) as wp,
            tc.tile_pool(name="enp", bufs=2 * GI) as enp,
            tc.tile_pool(name="etp", bufs=12) as etp,
            tc.tile_pool(name="tmp", bufs=3) as tmpp,
            tc.tile_pool(name="ps", bufs=8, space="PSUM") as ps,
        ):
            # ---- persistent loads ---------------------------------------
            def load2(name, dram, cols, dtt=dt.bfloat16):
                t = wp.tile([128, 2, cols], dtt, tag=name)
                nc.sync.dma_start(t[:], dram[:].rearrange("(cc p) x -> p cc x", p=128))
                return t

            wq = load2("wq", WQ, INNER)
            wk = load2("wk", WK, INNER)
            wkvwo = load2("wkvwo", WKVWO, H * DIM)
            wewo = load2("wewo", WEWO, H * DIM)
            nodt = load2("nodt", NODT, B * N)
            nodtm = load2("nodtm", NODTM, ROWS)
            bqt = wp.tile([128, 2, 1], dt.float32, tag="bqt")
            nc.sync.dma_start(bqt[:], BQ[:].rearrange("(cc p) x -> p cc x", p=128))
            wetd = wp.tile([DH, H * DIM], dt.bfloat16, tag="wetd")
            nc.sync.dma_start(wetd[:], WETD[:])
            biasf = wp.tile([1, DIM], dt.bfloat16, tag="biasf")
            nc.sync.dma_start(biasf[:], BIASF[:])
            idn = wp.tile([128, 128], dt.bfloat16, tag="idn")
            nc.sync.dma_start(idn[:], IDN[:])
            zeros = wp.tile([128, 128], dt.bfloat16, tag="zeros")
            nc.vector.memset(zeros[:], 0.0)
            onesc = wp.tile([128, 1], dt.bfloat16, tag="onesc")
            nc.sync.dma_start(onesc[:], ONESC[:])
            ones1f = wp.tile([1, 128], dt.float32, tag="ones1f")
            nc.sync.dma_start(ones1f[:], ONES1F[:])
            ones64 = wp.tile([1, NI], dt.bfloat16, tag="ones64")
            nc.sync.dma_start(ones64[:], ONES64[:])

            # ---- persistent compute buffers -----------------------------
            qtd = wp.tile([DH, H * ROWS], dt.bfloat16, tag="qtd")      # [d, h*128+i]
            ktd = wp.tile([DH, H * B * N], dt.bfloat16, tag="ktd")     # [d, h*1024+(b*512+j)]
            qwt = wp.tile([128, 2, H * ROWS], dt.bfloat16, tag="qwt")  # [c_p, cc, h*128+i]
            vwo = wp.tile([128, B * NT * H * DIM], dt.bfloat16, tag="vwo")
            ebuf = wp.tile([128, NT, NI * H], dt.bfloat16, tag="ebuf")   # per-b E
            simk = wp.tile([128, NT, NI * H], dt.bfloat16, tag="simk")   # per-b q.k
            atb = wp.tile([128, 2, NI * H], dt.bfloat16, tag="atb")      # per-b aE^T
            rbuf = wp.tile([1, NI * H], dt.float32, tag="rbuf")
            qsb = wp.tile([128, 2, 128], dt.bfloat16, tag="qsb")
            outsb = wp.tile([NI, DIM], dt.float32, tag="outsb")

            # ---- Stage A: projections -----------------------------------
            # qT [inner, i] then scatter to d-major qtd
            for mc in range(2):
                qp = ps.tile([128, 128], dt.float32, tag="ps")
                chain([
                    nc.tensor.matmul(
                        qp[:], wq[:, cc, mc * 128:(mc + 1) * 128],
                        nodtm[:, cc, :], start=(cc == 0), stop=(cc == 1),
                    )
                    for cc in range(2)])
                nc.vector.tensor_scalar_add(qsb[:, mc, :], qp[:], bqt[:, mc, :])
                for hq in range(4):
                    h = mc * 4 + hq
                    nc.vector.tensor_copy(
                        qtd[0:DH, h * ROWS:(h + 1) * ROWS],
                        qsb[hq * DH:(hq + 1) * DH, mc, :],
                    )
            # kT: no bias (cancels in softmax); scatter to d-major ktd
            for mc in range(2):
                for nch in range(2):
                    kp = ps.tile([128, 512], dt.float32, tag="ps")
                    chain([
                        nc.tensor.matmul(
                            kp[:], wk[:, cc, mc * 128:(mc + 1) * 128],
                            nodt[:, cc, nch * 512:(nch + 1) * 512],
                            start=(cc == 0), stop=(cc == 1),
                        )
                        for cc in range(2)])
                    for hq in range(4):
                        h = mc * 4 + hq
                        nc.vector.tensor_copy(
                            ktd[0:DH, h * 1024 + nch * 512: h * 1024 + (nch + 1) * 512],
                            kp[hq * DH:(hq + 1) * DH, :],
                        )
            # qW^T [c, (h,i)]
            for h in range(H):
                for cc in range(2):
                    qwp = ps.tile([128, 128], dt.float32, tag="ps")
                    nc.tensor.matmul(
                        qwp[:], wetd[0:DH, h * DIM + cc * 128: h * DIM + (cc + 1) * 128],
                        qtd[0:DH, h * ROWS:(h + 1) * ROWS], start=True, stop=True,
                    )
                    nc.vector.tensor_copy(qwt[:, cc, h * ROWS:(h + 1) * ROWS], qwp[:])
            # vWo [j, (b,t,h,m)]
            for jc in range(B * NT):
                for h in range(H):
                    vp = ps.tile([128, DIM], dt.float32, tag="ps")
                    chain([
                        nc.tensor.matmul(
                            vp[:], nodt[:, cc, jc * 128:(jc + 1) * 128],
                            wkvwo[:, cc, h * DIM:(h + 1) * DIM],
                            start=(cc == 0), stop=(cc == 1),
                        )
                        for cc in range(2)])
                    nc.any.tensor_copy(
                        vwo[:, (jc * H + h) * DIM:(jc * H + h + 1) * DIM], vp[:]
                    )

            # ---- Stage B: per-batch main loop ---------------------------
            # Column layouts: simk/ebuf/atb are h-major (col = h*64 + i_local);
            # skg/sadd are i-major within a group (col = il*8 + h).
            for b in range(2):
                # simk[j, h*64+i] per j-tile t
                for t in range(NT):
                    skp = ps.tile([128, NI * H], dt.float32, tag="ps")
                    chain([
                        nc.tensor.matmul(
                            skp[:, h * NI:(h + 1) * NI],
                            ktd[0:DH, h * 1024 + b * 512 + t * 128: h * 1024 + b * 512 + (t + 1) * 128],
                            qtd[0:DH, h * ROWS + b * NI: h * ROWS + (b + 1) * NI],
                            start=(h == 0), stop=(h == H - 1),
                        )
                        for h in range(H)])
                    nc.vector.tensor_copy(simk[:, t, :], skp[:])

                for g in range(NG):
                    # edge tiles for this group's 8 i's
                    ents, etts = [], []
                    for il in range(GI):
                        ig = b * NI + g * GI + il     # row in this core's 128
                        ent = enp.tile([128, NT, DIM], dt.bfloat16, tag="en")
                        nc.sync.dma_start(
                            ent[:], E_NAT[ig, :, :].rearrange("(t p) c -> p t c", p=128)
                        )
                        ett = etp.tile([128, 2, N], dt.bfloat16, tag="et")
                        nc.sync.dma_start(
                            ett[:], E_TRP[ig, :, :].rearrange("(cc p) j -> p cc j", p=128)
                        )
                        ents.append(ent)
                        etts.append(ett)

                    # sim_e: accumulate q.e into skg[j, t, il*8+h]
                    skg = ps.tile([128, NT, GI * H], dt.float32, tag="ps")
                    nmm = GI * NT * 2
                    mms = []
                    for il in range(GI):
                        ig = b * NI + g * GI + il
                        for t in range(NT):
                            for cc in range(2):
                                mms.append(nc.tensor.matmul(
                                    skg[:, t, il * H:(il + 1) * H],
                                    etts[il][:, cc, t * 128:(t + 1) * 128],
                                    qwt[:, cc, ig::ROWS],
                                    start=(len(mms) == 0), stop=(len(mms) == nmm - 1),
                                ))
                    chain(mms)

                    # exp(scale*(sim_e + sim_k)) -> ebuf (bf16, h-major cols)
                    for t in range(NT):
                        sadd = tmpp.tile([128, GI * H], dt.float32, tag="sadd")
                        nc.vector.tensor_add(
                            sadd[:].rearrange("p (i h) -> p i h", i=GI),
                            skg[:, t, :].rearrange("p (i h) -> p i h", i=GI),
                            simk[:, t, :].rearrange("p (h i) -> p i h", h=H)[:, g * GI:(g + 1) * GI, :],
                        )
                        nc.scalar.activation(
                            ebuf[:, t, :].rearrange("p (h i) -> p i h", h=H)[:, g * GI:(g + 1) * GI, :],
                            sadd[:].rearrange("p (i h) -> p i h", i=GI),
                            AF.Exp, scale=SCALE,
                        )

                    # denominators and reciprocal (cols ordered h*GI+il)
                    dg = ps.tile([1, GI * H], dt.float32, tag="ps")
                    mms = []
                    for t in range(NT):
                        for h in range(H):
                            mms.append(nc.tensor.matmul(
                                dg[:, h * GI:(h + 1) * GI],
                                onesc[:],
                                ebuf[:, t, h * NI + g * GI: h * NI + (g + 1) * GI],
                                start=(t == 0 and h == 0),
                                stop=(t == NT - 1 and h == H - 1),
                            ))
                    chain(mms)
                    nc.vector.reciprocal(rbuf[:, g * GI * H:(g + 1) * GI * H], dg[:])
                    # broadcast recip across partitions (f32 outer product)
                    rg = ps.tile([128, GI * H], dt.float32, tag="ps")
                    nc.tensor.matmul(
                        rg[:], ones1f[:], rbuf[:, g * GI * H:(g + 1) * GI * H],
                        start=True, stop=True,
                    )
                    # normalize E in place (rg cols ordered h*GI+il)
                    for t in range(NT):
                        nc.vector.tensor_mul(
                            ebuf[:, t, :].rearrange("p (h i) -> p h i", h=H)[:, :, g * GI:(g + 1) * GI],
                            ebuf[:, t, :].rearrange("p (h i) -> p h i", h=H)[:, :, g * GI:(g + 1) * GI],
                            rg[:].rearrange("p (h i) -> p h i", h=H),
                        )

                    # aE: quads of 4 i's packed at partition offsets 0/32/64/96.
                    # Each MM is M=32: rows 0:8 of the slot are this i's aE; rows
                    # 8:32 are junk products (neighbor i columns vs this i's edges)
                    # that keep the whole bank initialized and are never read.
                    for q2 in range(2):
                        aq = ps.tile([128, DIM], dt.float32, tag="ps")
                        mms = [nc.tensor.matmul(
                            aq[:], zeros[:], nodt[:, 0, 0:DIM],
                            start=True, stop=False)]
                        for t in range(NT):
                            for qi in range(4):
                                il = q2 * 4 + qi
                                i0 = g * GI + il
                                lhs = ebuf[:, t, :].rearrange(
                                    "p (h i) -> p i h", h=H)[:, i0, :]
                                mms.append(nc.tensor.matmul(
                                    aq[qi * 32:qi * 32 + H, :],
                                    lhs,
                                    ents[il][:, t, :],
                                    tile_position=(0, qi * 32),
                                    start=False, stop=False,
                                ))
                        # full-bank closing matmul (adds zero, clears group state)
                        mms.append(nc.tensor.matmul(
                            aq[:], zeros[:], nodt[:, 0, 0:DIM],
                            start=False, stop=True))
                        chain(mms)
                        asb = tmpp.tile([128, DIM], dt.bfloat16, tag="asb")
                        nc.vector.tensor_copy(asb[:], aq[:])
                        for cc in range(2):
                            tp = ps.tile([128, 128], dt.bfloat16, tag="ps")
                            nc.tensor.transpose(tp[:], asb[:, cc * 128:(cc + 1) * 128], idn[:])
                            # gather the 4x8 live rows -> atb columns (h-major)
                            src = tp[:, 0:128].rearrange("p (q x) -> p q x", q=4)[:, :, 0:H]
                            dst = atb[:, cc, :].rearrange("p (h i) -> p i h", h=H)[
                                :, g * GI + q2 * 4: g * GI + q2 * 4 + 4, :]
                            nc.vector.tensor_copy(dst, src)

                # ---- per-batch tail: final output ----------------------
                outp = ps.tile([NI, DIM], dt.float32, tag="ps")
                mms = [nc.tensor.matmul(outp[:], ones64[:], biasf[:], start=True, stop=False)]
                for h in range(H):
                    for t in range(NT):
                        mms.append(nc.tensor.matmul(
                            outp[:], ebuf[:, t, h * NI:(h + 1) * NI],
                            vwo[:, ((b * NT + t) * H + h) * DIM:((b * NT + t) * H + h + 1) * DIM],
                            start=False, stop=False,
                        ))
                for h in range(H):
                    for cc in range(2):
                        mms.append(nc.tensor.matmul(
                            outp[:], atb[:, cc, h * NI:(h + 1) * NI],
                            wewo[:, cc, h * DIM:(h + 1) * DIM],
                            start=False, stop=(h == H - 1 and cc == 1),
                        ))
                chain(mms)
                nc.vector.tensor_copy(outsb[:], outp[:])
                nc.sync.dma_start(OUT[b * NI:(b + 1) * NI, :], outsb[:])

    nc.compile()
    return nc


def _prep_inputs(nodes, edges, Wq, bq, Wkv, bkv, We, be, Wo, bo):
    """Host-side preprocessing: dtype conversion, layout transforms, weight folding."""
    f32 = np.float32
    nodes = np.asarray(nodes, f32)
    Wq = np.asarray(Wq, f32); bq = np.asarray(bq, f32)
    Wkv = np.asarray(Wkv, f32); bkv = np.asarray(bkv, f32)
    We = np.asarray(We, f32); be = np.asarray(be, f32)
    Wo = np.asarray(Wo, f32); bo = np.asarray(bo, f32)

    nodt = np.ascontiguousarray(
        nodes.reshape(B * N, DIM).T).astype(bf16)                      # [c, b*512+n]
    Wk = Wkv[:, :INNER]
    Wv = Wkv[:, INNER:]
    wetd = np.ascontiguousarray(
        We.reshape(DIM, H, DH).transpose(2, 1, 0)).reshape(DH, H * DIM).astype(bf16)
    wkvwo = np.einsum(
        "chd,hdm->chm", Wv.reshape(DIM, H, DH), Wo.reshape(H, DH, DIM)
    ).reshape(DIM, H * DIM).astype(bf16)
    wewo = np.einsum(
        "chd,hdm->chm", We.reshape(DIM, H, DH), Wo.reshape(H, DH, DIM)
    ).reshape(DIM, H * DIM).astype(bf16)
    biasf = (bo + (be + bkv[INNER:]) @ Wo).reshape(1, DIM).astype(bf16)

    common = {
        "nodt": nodt,
        "wq": Wq.astype(bf16),
        "wk": Wk.astype(bf16),
        "wetd": wetd,
        "wkvwo": wkvwo,
        "wewo": wewo,
        "bq": bq.reshape(INNER, 1).astype(f32),
        "biasf": biasf,
        "idn": np.eye(128, dtype=f32).astype(bf16),
        "onesc": np.ones((128, 1), f32).astype(bf16),
        "ones1f": np.ones((1, 128), f32),
        "ones64": np.ones((1, NI), f32).astype(bf16),
    }

    edges_bf = np.asarray(edges, f32).astype(bf16)   # (B, N, N, DIM)
    in_maps = []
    for c in range(NCORES):
        sl = edges_bf[:, c * NI:(c + 1) * NI]        # (B, NI, N, DIM)
        e_nat = np.ascontiguousarray(sl.reshape(ROWS, N, DIM))
        e_trp = np.ascontiguousarray(e_nat.transpose(0, 2, 1))
        nm = nodes.reshape(B, N, DIM)[:, c * NI:(c + 1) * NI].reshape(ROWS, DIM)
        nodtm = np.ascontiguousarray(nm.T).astype(bf16)
        in_maps.append({**common, "e_nat": e_nat, "e_trp": e_trp, "nodtm": nodtm})
    return in_maps


def kernel(**inputs):
    from concourse import bass_utils

    if "nc" not in _CACHE:
        _CACHE["nc"] = _build_core()
    nc = _CACHE["nc"]

    in_maps = _prep_inputs(**inputs)
    res = bass_utils.run_bass_kernel_spmd(nc, in_maps, core_ids=list(range(NCORES)))

    out = np.empty((B, N, DIM), np.float32)
    for c in range(NCORES):
        o = res.results[c]["out"]                    # [ROWS, DIM]
        out[:, c * NI:(c + 1) * NI] = o.reshape(B, NI, DIM)
    return out
